# revision 24
# baseline (speedup 1.0000x reference)
"""Char-LSTM kernel for Trainium2 (8 NeuronCores, data parallel).

Strategy
--------
Host side (all pure preprocessing of weights + input layout):
  * The LSTM state after the first one/two characters is a function of the
    weights only: precompute f32 tables (h1,c1)[100] (one char) and
    (h2,c2)[100*100] (two chars).  Words of length 1/2 are answered straight
    from the tables; every other word starts on-device from a table state,
    so a length-L word runs only L-2 device steps.
  * Words are binned into 14 "tiers" by effective (remaining) length
    Le = L-2 (or L-1 via the one-char table when a tier needs filling).
    Each tier is exactly 4096 words = 8 cores x 512 columns.  The handful
    of words that fit no tier (~100-200, counts are random) are finished on
    the host.  The device program is therefore FIXED: 7 block-pairs per
    core, 56 recurrence steps, independent of the data.
  * Per step the device consumes x = emb[char] (rank-32 trick: the 4Hx100
    one-hot embedding matmul of the usual formulation collapses to a
    K=32 matmul against W_ih^T, plus a ones-row for the bias).

Device side (identical SPMD program on all 8 cores):
  * Layout: two 512-word blocks A,B share every tile: A in partitions 0:64,
    B in 64:128 (a partition = one hidden dim of one block).  Per step and
    gate bank q, ONE M=128 matmul with a block-diagonal lhsT computes the
    bank for A and B words at once (the baseline needed two M=64 matmuls).
  * Gates are banked (i | f | o | 2g) across two PSUM tiles per step; the
    g-bank weights/bias are pre-scaled by 2 so a single Sigmoid over each
    PSUM tile covers everything (tanh g = 2*sigmoid(2g)-1, fixed up in the
    cell update below).
  * Cell update on DVE (bf16 in SBUF => 2x mode):
        v  = (sig2g - 0.5) * sigi          # scalar_tensor_tensor
        m  = sigf * c
        c' = 2*v + m                       # scalar_tensor_tensor
        h' = sigo * tanh(c')               # tanh on ACT
  * Final h per block is DMA'd out in bf16 at that block's last step.
"""

import os
import sys

for _p in ("/opt/trn_rl_repo", "/root/.axon_site/_ro/trn_rl_repo"):
    if os.path.isdir(_p) and _p not in sys.path:
        sys.path.insert(0, _p)

import numpy as np
import ml_dtypes

BF16 = ml_dtypes.bfloat16

H = 64
E = 32
V = 100
NCORES = 8
BLK = 512           # words per block (columns)
NTIERS = 14         # effective lengths 1..14
GATE4 = 4 * H

# torch gate order in W/b is [i, f, g, o] (chunks of 64). Device bank order
# is (i, f, o, g); the g bank is scaled by 2 for the sigmoid-only trick.
_BANKS = [np.arange(0, 64), np.arange(64, 128), np.arange(192, 256),
          np.arange(128, 192)]

_PROGRAM_CACHE = {}
SCHED_WIDTH = int(os.environ.get("LSTM_WIDTH", "4"))
C_BF16 = os.environ.get("LSTM_CBF16", "0") == "1"
PS_BIG = os.environ.get("LSTM_PSBIG", "1") == "1"
T_PAIR = os.environ.get("LSTM_TPAIR", "0") == "1"


def _sigmoid(x):
    return 1.0 / (1.0 + np.exp(-x))


def _host_step(h, c, x, W_ih, W_hh, b):
    """One LSTM step in f32 numpy; h,c,x: [N, *]."""
    gates = x @ W_ih.T + h @ W_hh.T + b
    i = _sigmoid(gates[:, 0:64])
    f = _sigmoid(gates[:, 64:128])
    g = np.tanh(gates[:, 128:192])
    o = _sigmoid(gates[:, 192:256])
    c2 = f * c + i * g
    h2 = o * np.tanh(c2)
    return h2, c2


def _prefix_tables(emb, W_ih, W_hh, b):
    """(h1,c1)[100], (h2,c2)[10000] — LSTM state after 1 / 2 chars."""
    z = np.zeros((V, H), np.float32)
    h1, c1 = _host_step(z, z, emb, W_ih, W_hh, b)
    h1r = np.repeat(h1, V, axis=0)            # index = c0*100 + c1
    c1r = np.repeat(c1, V, axis=0)
    x2 = np.tile(emb, (V, 1))
    h2, c2 = _host_step(h1r, c1r, x2, W_ih, W_hh, b)
    return h1, c1, h2, c2


# --------------------------------------------------------------------------
# Planning
# --------------------------------------------------------------------------

def _plan(lengths):
    """Tier assignment.

    Returns:
      tiers: dict Le -> (words[4096] int64 (-1 dummy), start_off[4096] int8)
             start_off: 2 => two-char table, 1 => one-char table, 0 => dummy
      host_full: word ids (len>=3) the host must run fully
      host1, host2: word ids answered from the h1 / h2 tables
    """
    lengths = np.asarray(lengths).astype(np.int64)
    host1 = np.nonzero(lengths == 1)[0]
    host2 = np.nonzero(lengths == 2)[0]

    by_len = {L: list(np.nonzero(lengths == L)[0]) for L in range(3, 17)}
    tiers = {}
    carry = []          # len == Le+1 words overflowed from the tier below
    stuck = []
    for Le in range(1, NTIERS + 1):
        words, offs = [], []
        # carried words (len == Le+1) run here via the one-char table —
        # this tier is their last chance.
        take = carry[:4096]
        stuck += carry[len(take):]
        words += take
        offs += [1] * len(take)
        nat = by_len.get(Le + 2, [])
        room = 4096 - len(words)
        words += nat[:room]
        offs += [2] * len(nat[:room])
        carry = nat[room:]
        w = np.full(4096, -1, np.int64)
        o = np.zeros(4096, np.int8)
        w[:len(words)] = words
        o[:len(offs)] = offs
        tiers[Le] = (w, o)
    host_full = np.array(sorted(stuck + carry), dtype=np.int64)
    return {"tiers": tiers, "host_full": host_full,
            "host1": host1, "host2": host2}


# Fixed group structure: (tierA, tierB) pairs; steps = LeA.
_GROUPS = [(14, 13), (12, 11), (10, 9), (8, 7), (6, 5), (4, 3), (2, 1)]


def _schedule_rounds(width=None):
    """width-wide interleave; an admitted group runs to completion (paused
    groups would pin live state tiles and deadlock the tile pools).
    Returns a list of rounds, each a list of (group, t)."""
    width = SCHED_WIDTH if width is None else width
    remaining = {g: a for g, (a, _) in enumerate(_GROUPS)}
    next_t = [0] * len(_GROUPS)
    queue = sorted(remaining, key=lambda g: -remaining[g])
    running = []
    rounds = []
    while queue or running:
        while len(running) < width and queue:
            running.append(queue.pop(0))
        rnd = []
        for g in list(running):
            rnd.append((g, next_t[g]))
            next_t[g] += 1
            remaining[g] -= 1
            if remaining[g] == 0:
                running.remove(g)
        rounds.append(rnd)
    return rounds


def _schedule(width=None):
    return [e for rnd in _schedule_rounds(width) for e in rnd]


# --------------------------------------------------------------------------
# Device program
# --------------------------------------------------------------------------

def _build_program(reps=1):
    import concourse.bass as bass  # noqa: F401 (registers engines)
    import concourse.tile as tile
    from concourse import bacc, mybir
    from contextlib import nullcontext

    f32 = mybir.dt.float32
    bf16 = mybir.dt.bfloat16
    SIG = mybir.ActivationFunctionType.Sigmoid
    TANH = mybir.ActivationFunctionType.Tanh
    SUB = mybir.AluOpType.subtract
    XBATCH = 4
    MUL = mybir.AluOpType.mult
    ADD = mybir.AluOpType.add

    sched = _schedule()
    n_slabs = len(sched)

    nc = bacc.Bacc("TRN2", target_bir_lowering=False, debug=False,
                   num_devices=NCORES)
    w_d = nc.dram_tensor("w", [128, GATE4 * 4], bf16, kind="ExternalInput")
    x_d = nc.dram_tensor("xs", [66, n_slabs * BLK], bf16, kind="ExternalInput")
    hinit_d = nc.dram_tensor("hinit", [128, len(_GROUPS) * BLK], bf16,
                             kind="ExternalInput")
    cdt = bf16 if C_BF16 else f32
    cinit_d = nc.dram_tensor("cinit", [128, len(_GROUPS) * BLK], cdt,
                             kind="ExternalInput")
    out_d = nc.dram_tensor("out", [NTIERS, H, BLK], bf16,
                           kind="ExternalOutput")

    with tile.TileContext(nc) as tc:
        with (
            tc.tile_pool(name="consts", bufs=1) as consts,
            tc.tile_pool(name="xs", bufs=10) as xpool,
            tc.tile_pool(name="ps", bufs=(2 if PS_BIG else 4), space="PSUM") as pspool,
            tc.tile_pool(name="sig", bufs=8) as sigpool,
            tc.tile_pool(name="vt", bufs=4) as vpool,
            tc.tile_pool(name="mt", bufs=4) as mpool,
            tc.tile_pool(name="ct", bufs=6) as cpool,
            tc.tile_pool(name="tc_", bufs=4) as tcpool,
            tc.tile_pool(name="ht", bufs=6) as hpool,
            tc.tile_pool(name="h0", bufs=1) as h0pool,
            tc.tile_pool(name="c0", bufs=1) as c0pool,
        ):
            wb = consts.tile([128, GATE4 * 4], bf16, tag="wb")
            nc.scalar.dma_start(out=wb[:], in_=w_d[:])
            wh = wb[:, 0:GATE4 * 2]
            wx = wb[0:66, GATE4 * 2:GATE4 * 4]

            loop_cm = tc.For_i(0, reps, 1) if reps > 1 else nullcontext()
            with loop_cm:
                state = {}
                # Prefetch every group's initial state in two bulk DMAs so a
                # newly admitted group never stalls the pipeline.
                ng = len(_GROUPS)
                h0 = h0pool.tile([128, ng * BLK], bf16, tag="h0")
                c0 = c0pool.tile([128, ng * BLK], cdt, tag="c0")
                half = (SCHED_WIDTH * BLK if SCHED_WIDTH < ng else ng * BLK)
                nc.gpsimd.dma_start(out=h0[:, 0:half], in_=hinit_d[:, 0:half])
                nc.gpsimd.dma_start(out=c0[:, 0:half], in_=cinit_d[:, 0:half])
                if half < ng * BLK:
                    nc.gpsimd.dma_start(out=h0[:, half:], in_=hinit_d[:, half:])
                    nc.gpsimd.dma_start(out=c0[:, half:], in_=cinit_d[:, half:])
                for g in range(ng):
                    state[g] = (h0[:, g * BLK:(g + 1) * BLK],
                                c0[:, g * BLK:(g + 1) * BLK])
                rounds = _schedule_rounds()
                slab_idx = 0
                chunk = 2 if T_PAIR else 1
                parts = [rnd[ci:ci + chunk] for rnd in rounds
                         for ci in range(0, len(rnd), chunk)]
                for part in parts:
                  np_ = len(part)
                  csup = cpool.tile([128, chunk * BLK], cdt, tag="ct",
                                    name="c2")
                  tsup = tcpool.tile([128, chunk * BLK], bf16, tag="tc")
                  outs = []
                  for k, (g, t) in enumerate(part):
                    LeA, LeB = _GROUPS[g]

                    xst = xpool.tile([66, BLK], bf16, tag="xs")
                    nc.sync.dma_start(
                        out=xst[:],
                        in_=x_d[:, slab_idx * BLK:(slab_idx + 1) * BLK])
                    xs = xst[:]
                    slab_idx += 1

                    h, c = state[g]

                    mm = nc.tensor.matmul
                    if PS_BIG:
                        # one PSUM tile, banks (i | 2g | f | o)
                        ps = pspool.tile([128, 4 * BLK], f32, tag="ps")
                        regions = [ps[:, 0:BLK], ps[:, BLK:2 * BLK],
                                   ps[:, 2 * BLK:3 * BLK], ps[:, 3 * BLK:4 * BLK]]
                    else:
                        # two PSUM tiles, banks (i | 2g) and (f | o).
                        ps_ig = pspool.tile([128, 2 * BLK], f32, tag="ps")
                        ps_fo = pspool.tile([128, 2 * BLK], f32, tag="ps")
                        regions = [ps_ig[:, 0:BLK], ps_ig[:, BLK:2 * BLK],
                                   ps_fo[:, 0:BLK], ps_fo[:, BLK:2 * BLK]]
                    # x-part first (start=True clears the bank), recurrent
                    # part second (stop=True closes the accumulation group).
                    wxs = [wx[:, 0:128], wx[:, 384:512], wx[:, 128:256], wx[:, 256:384]]
                    whs = [wh[:, 0:128], wh[:, 384:512], wh[:, 128:256], wh[:, 256:384]]
                    for r, w_ in zip(regions, wxs):
                        mm(r, w_, xs, start=True, stop=False)
                    for r, w_ in zip(regions, whs):
                        mm(r, w_, h[:], start=False, stop=True)

                    if PS_BIG:
                        s = sigpool.tile([128, 4 * BLK], bf16, tag="sig")
                        nc.scalar.activation(out=s[:], in_=ps[:], func=SIG)
                        s_i, s_2g = s[:, 0:BLK], s[:, BLK:2 * BLK]
                        s_f, s_o = s[:, 2 * BLK:3 * BLK], s[:, 3 * BLK:4 * BLK]
                    else:
                        s_ig = sigpool.tile([128, 2 * BLK], bf16, tag="sig")
                        s_fo = sigpool.tile([128, 2 * BLK], bf16, tag="sig")
                        nc.scalar.activation(out=s_ig[:], in_=ps_ig[:], func=SIG)
                        nc.scalar.activation(out=s_fo[:], in_=ps_fo[:], func=SIG)
                        s_i, s_2g = s_ig[:, 0:BLK], s_ig[:, BLK:2 * BLK]
                        s_f, s_o = s_fo[:, 0:BLK], s_fo[:, BLK:2 * BLK]

                    # v = (sig2g - 0.5) * sigi ; c' = 2v + sigf*c
                    v = vpool.tile([128, BLK], bf16, tag="vt")
                    nc.vector.scalar_tensor_tensor(v[:], s_2g, 0.5, s_i,
                                                   SUB, MUL)
                    m = mpool.tile([128, BLK], cdt, tag="mt")
                    nc.vector.tensor_mul(m[:], s_f, c[:])
                    c2 = csup[:, k * BLK:(k + 1) * BLK]
                    nc.vector.scalar_tensor_tensor(c2, v[:], 2.0, m[:],
                                                   MUL, ADD)
                    outs.append((g, t, s_o, c2))

                  # one tanh for the whole chunk (pairs chains when T_PAIR)
                  nc.scalar.activation(out=tsup[:, 0:np_ * BLK],
                                       in_=csup[:, 0:np_ * BLK], func=TANH)
                  for k, (g, t, s_o, c2) in enumerate(outs):
                    LeA, LeB = _GROUPS[g]
                    tch = tsup[:, k * BLK:(k + 1) * BLK]
                    h2 = hpool.tile([128, BLK], bf16, tag="ht", name="h2")
                    nc.vector.tensor_mul(h2[:], s_o, tch)
                    state[g] = (h2, c2)

                    if t == LeA - 1:
                        nc.sync.dma_start(out=out_d[LeA - 1], in_=h2[0:64, :])
                    if t == LeB - 1:
                        nc.sync.dma_start(out=out_d[LeB - 1],
                                          in_=h2[64:128, :])

    nc.compile()
    return nc


# --------------------------------------------------------------------------
# Host data packing
# --------------------------------------------------------------------------

def _pack_weights(W_ih, W_hh, b):
    wh = np.zeros((128, GATE4 * 2), np.float32)
    wx = np.zeros((66, GATE4 * 2), np.float32)
    for q, rows in enumerate(_BANKS):
        s = 2.0 if q == 3 else 1.0
        WhT = W_hh[rows].T * s          # [64, 64]
        WxT = W_ih[rows].T * s          # [32, 64]
        bq = b[rows] * s
        wh[0:64, 128 * q:128 * q + 64] = WhT
        wh[64:128, 128 * q + 64:128 * q + 128] = WhT
        wx[0:32, 128 * q:128 * q + 64] = WxT
        wx[32, 128 * q:128 * q + 64] = bq
        wx[33:65, 128 * q + 64:128 * q + 128] = WxT
        wx[65, 128 * q + 64:128 * q + 128] = bq
    w = np.zeros((128, GATE4 * 4), np.float32)
    w[:, 0:GATE4 * 2] = wh
    w[0:66, GATE4 * 2:GATE4 * 4] = wx
    return w.astype(BF16)


def _pack_core_data(plan, chars, emb_bf, h1, c1, h2, c2, core):
    """Build x_d, hinit_d, cinit_d for one core. Also returns the per-tier
    word/column assignment for unpacking."""
    sched = _schedule()
    tiers = plan["tiers"]
    lo, hi = core * BLK, (core + 1) * BLK

    assign = {}
    for Le in range(1, NTIERS + 1):
        w, o = tiers[Le]
        assign[Le] = (w[lo:hi], o[lo:hi])

    n_slabs = len(sched)
    x = np.zeros((n_slabs, 66, BLK), BF16)
    x[:, 32, :] = 1.0
    x[:, 65, :] = 1.0
    hinit = np.zeros((128, len(_GROUPS) * BLK), BF16)
    cinit = np.zeros((128, len(_GROUPS) * BLK), BF16 if C_BF16 else np.float32)

    for g, (LeA, LeB) in enumerate(_GROUPS):
        for half, Le in ((0, LeA), (1, LeB)):
            w, off = assign[Le]
            valid = w >= 0
            wv = w[valid]
            cols = np.nonzero(valid)[0]
            pref2 = None
            if wv.size:
                c0 = chars[wv, 0]
                two = off[valid] == 2
                pref2 = np.where(two, c0 * V + chars[wv, 1], 0)
                hrow = np.where(two[:, None], h2[pref2], h1[c0])
                crow = np.where(two[:, None], c2[pref2], c1[c0])
                hinit[64 * half:64 * half + 64, g * BLK + cols] = \
                    hrow.astype(BF16).T
                cinit[64 * half:64 * half + 64, g * BLK + cols] = \
                    crow.astype(cinit.dtype).T
        # x slabs filled below per sched entry

    for slab_idx, (g, t) in enumerate(sched):
        LeA, LeB = _GROUPS[g]
        for half, Le in ((0, LeA), (1, LeB)):
            if t >= Le:
                continue
            w, off = assign[Le]
            valid = w >= 0
            wv = w[valid]
            if not wv.size:
                continue
            cols = np.nonzero(valid)[0]
            ch = chars[wv, off[valid].astype(np.int64) + t]
            x[slab_idx, 33 * half:33 * half + 32, cols] = emb_bf[ch]
    x = np.ascontiguousarray(x.transpose(1, 0, 2).reshape(66, n_slabs * BLK))
    return x, hinit, cinit, assign


# --------------------------------------------------------------------------
# Entry point
# --------------------------------------------------------------------------

def kernel(emb, W_ih, W_hh, b_ih, b_hh, chars, lengths):
    from concourse.bass_utils import run_bass_kernel_spmd

    emb = np.asarray(emb, dtype=np.float32)
    W_ih = np.asarray(W_ih, dtype=np.float32)
    W_hh = np.asarray(W_hh, dtype=np.float32)
    b = (np.asarray(b_ih, dtype=np.float32)
         + np.asarray(b_hh, dtype=np.float32))
    chars = np.asarray(chars)
    lengths_np = np.asarray(lengths)
    n = chars.shape[0]

    h1, c1, h2, c2 = _prefix_tables(emb, W_ih, W_hh, b)
    w = _pack_weights(W_ih, W_hh, b)
    emb_bf = emb.astype(BF16)

    plan = _plan(lengths_np)

    if "prog" not in _PROGRAM_CACHE:
        _PROGRAM_CACHE["prog"] = _build_program()
    nc = _PROGRAM_CACHE["prog"]

    in_maps = []
    assigns = []
    for core in range(NCORES):
        x, hinit, cinit, assign = _pack_core_data(
            plan, chars, emb_bf, h1, c1, h2, c2, core)
        in_maps.append({"w": w, "xs": x,
                        "hinit": hinit, "cinit": cinit})
        assigns.append(assign)

    res = run_bass_kernel_spmd(nc, in_maps, core_ids=list(range(NCORES)))
    kernel._last_nc = nc
    kernel._last_in_maps = in_maps

    result = np.empty((n, H), dtype=np.float32)

    # device words
    for core in range(NCORES):
        out = res.results[core]["out"]          # [14, 64, 512] bf16
        for Le in range(1, NTIERS + 1):
            w, _ = assigns[core][Le]
            valid = w >= 0
            if not valid.any():
                continue
            cols = np.nonzero(valid)[0]
            result[w[valid]] = out[Le - 1][:, cols].T.astype(np.float32)

    # host words
    h1w = plan["host1"]
    if h1w.size:
        result[h1w] = h1[chars[h1w, 0]]
    h2w = plan["host2"]
    if h2w.size:
        result[h2w] = h2[chars[h2w, 0] * V + chars[h2w, 1]]
    hf = plan["host_full"]
    if hf.size:
        hh = h2[chars[hf, 0] * V + chars[hf, 1]].copy()
        cc = c2[chars[hf, 0] * V + chars[hf, 1]].copy()
        L = lengths_np[hf]
        for t in range(2, int(L.max())):
            activef = t < L
            idx = np.nonzero(activef)[0]
            x = emb[chars[hf[idx], t]]
            hh[idx], cc[idx] = _host_step(hh[idx], cc[idx], x, W_ih, W_hh, b)
        result[hf] = hh

    return result
